# revision 1
# baseline (speedup 1.0000x reference)
"""GQA attention (SEQ=2048, DIM=4096, 32 Q heads / 8 KV heads, head_dim=128),
tensor-parallel over heads across 8 NeuronCores.

Each core owns 4 Q heads + 1 KV head: wq/wk/wv split column-wise, wo split
row-wise; each core produces a partial (2048, 4096) output that the host sums
(the all-reduce of row-parallel wo).

Per-core kernel (matmuls on the float32r PE path: full fp32 operand bytes,
tf32-like rounding, 1 cyc/row at free-dim 512 vs 4 cyc/row for plain fp32):
  A) QKV projections: stream xT (dim-major) blocks; Q^T/K^T/V^T accumulate in
     PSUM over the 4096 contraction; RoPE applied on PSUM eviction; V^T
     transposed back to V via PE transposes.
  B) Flash-style attention per (head, 512-query block): S^T = K^T_blk.T @ Q^T
     (keys on partitions), causal staircase mask added on diagonal blocks,
     exp on ACT (scale=1/sqrt(128) folded in), D = ones.T @ expS^T summed over
     key blocks on the PE, O^T = V_blk.T @ expS^T accumulated in PSUM,
     normalized by 1/D (PE broadcast of the reciprocal) on eviction.
  C) out = O^T.T @ wo accumulated over the 4 heads, streamed to DRAM.
"""

import numpy as np

import concourse.bacc as bacc
import concourse.tile as tile
from concourse import mybir
from concourse.bass_utils import run_bass_kernel_spmd

F32 = mybir.dt.float32
F32R = mybir.dt.float32r

DIM = 4096
SEQ = 2048
HEAD_DIM = 128
N_CORES = 8
QH = 4            # q heads per core
QS = QH * HEAD_DIM  # 512: wq column slice per core
NKT = DIM // 128    # 32 contraction tiles
NSB = SEQ // 512    # 4 sequence blocks
SCALE = 1.0 / float(np.sqrt(HEAD_DIM))
NEG = -1e9


def build_nc():
    nc = bacc.Bacc(trn_type="TRN2")

    xT = nc.declare_dram_parameter("xT", [DIM, SEQ], F32R, isOutput=False)
    wq = nc.declare_dram_parameter("wq", [DIM, QS], F32R, isOutput=False)
    wk = nc.declare_dram_parameter("wk", [DIM, HEAD_DIM], F32R, isOutput=False)
    wv = nc.declare_dram_parameter("wv", [DIM, HEAD_DIM], F32R, isOutput=False)
    wo = nc.declare_dram_parameter("wo", [QS, DIM], F32R, isOutput=False)
    cosT = nc.declare_dram_parameter("cosT", [HEAD_DIM, SEQ], F32, isOutput=False)
    sinTs = nc.declare_dram_parameter("sinTs", [HEAD_DIM, SEQ], F32, isOutput=False)
    stair = nc.declare_dram_parameter("stair", [128, 896], F32, isOutput=False)
    ident = nc.declare_dram_parameter("ident", [128, 128], F32R, isOutput=False)
    ones_col = nc.declare_dram_parameter("ones_col", [128, 1], F32R, isOutput=False)
    ones_row = nc.declare_dram_parameter("ones_row", [1, 128], F32R, isOutput=False)
    out = nc.declare_dram_parameter("out", [SEQ, DIM], F32, isOutput=True)

    with tile.TileContext(nc) as tc:
        with (
            tc.tile_pool(name="persist", bufs=1) as persist,
            tc.tile_pool(name="resid", bufs=1) as resid,
        ):
            # small constants
            stair_sb = persist.tile([128, 896], F32)
            nc.sync.dma_start(out=stair_sb, in_=stair[:, :])
            ident_sb = persist.tile([128, 128], F32R)
            nc.sync.dma_start(out=ident_sb, in_=ident[:, :])
            onesc_sb = persist.tile([128, 1], F32R)
            nc.sync.dma_start(out=onesc_sb, in_=ones_col[:, :])
            onesr_sb = persist.tile([1, 128], F32R)
            nc.sync.dma_start(out=onesr_sb, in_=ones_row[:, :])

            # resident activations
            qT = resid.tile([128, QH, SEQ], F32R)      # Q^T per head (d, seq)
            kT = resid.tile([128, SEQ], F32R)          # K^T (d, seq)
            vN = resid.tile([128, SEQ // 128, 128], F32R)  # V natural (keys, d)

            # ---------------- Phase A: projections + RoPE ----------------
            with (
                tc.tile_pool(name="wpool", bufs=1) as wpool,
                tc.tile_pool(name="xpool", bufs=2) as xpool,
                tc.tile_pool(name="cspool", bufs=2) as cspool,
                tc.tile_pool(name="ropetmp", bufs=2) as ropetmp,
                tc.tile_pool(name="vtb", bufs=2) as vtb,
                tc.tile_pool(name="psA", bufs=1, space="PSUM") as psA,
                tc.tile_pool(name="psVT", bufs=2, space="PSUM") as psVT,
            ):
                # resident weights: per-4kt chunk tiles so the first matmuls
                # only wait on their own 1 MiB DMA, not the whole weight load
                wq_r = wq.rearrange("(t p) m -> p t m", p=128)
                wk_r = wk.rearrange("(t p) m -> p t m", p=128)
                wv_r = wv.rearrange("(t p) m -> p t m", p=128)
                wq_cs, wk_cs, wv_cs = [], [], []
                for c in range(8):
                    wq_cs.append(wpool.tile([128, 4, QS], F32R, name=f"wqc{c}"))
                    wk_cs.append(wpool.tile([128, 4, HEAD_DIM], F32R, name=f"wkc{c}"))
                    wv_cs.append(wpool.tile([128, 4, HEAD_DIM], F32R, name=f"wvc{c}"))

                xT_r = xT.rearrange("(t p) s -> p t s", p=128)

                for sb in range(NSB):
                    ss = slice(sb * 512, (sb + 1) * 512)
                    # PSUM accumulators for this seq block
                    q_ps = [psA.tile([128, 512], F32, tag=f"qps{h}", name=f"qps{h}")
                            for h in range(QH)]
                    k_ps = psA.tile([128, 512], F32, tag="kps")
                    v_ps = psA.tile([128, 512], F32, tag="vps")

                    for g in range(8):  # super-tiles of 4 k-tiles (1 MiB DMAs)
                        if sb == 0:
                            # interleave weight chunks with the x stream so the
                            # first matmuls only queue behind ~2.5 MB of DMA
                            nc.sync.dma_start(
                                out=wq_cs[g], in_=wq_r[:, g * 4:(g + 1) * 4, :]
                            )
                            nc.sync.dma_start(
                                out=wk_cs[g], in_=wk_r[:, g * 4:(g + 1) * 4, :]
                            )
                            nc.sync.dma_start(
                                out=wv_cs[g], in_=wv_r[:, g * 4:(g + 1) * 4, :]
                            )
                        xt = xpool.tile([128, 4, 512], F32R, tag="xt")
                        nc.sync.dma_start(
                            out=xt, in_=xT_r[:, g * 4:(g + 1) * 4, ss]
                        )
                        for i in range(4):
                            kt = g * 4 + i
                            st = (kt == 0)
                            sp = (kt == NKT - 1)
                            for h in range(QH):
                                nc.tensor.matmul(
                                    q_ps[h],
                                    wq_cs[g][:, i, h * 128:(h + 1) * 128],
                                    xt[:, i, :],
                                    start=st, stop=sp,
                                )
                            nc.tensor.matmul(
                                k_ps, wk_cs[g][:, i, :], xt[:, i, :],
                                start=st, stop=sp,
                            )
                            nc.tensor.matmul(
                                v_ps, wv_cs[g][:, i, :], xt[:, i, :],
                                start=st, stop=sp,
                            )

                    # RoPE tables for this block
                    cos_t = cspool.tile([128, 512], F32, tag="cos")
                    nc.sync.dma_start(out=cos_t, in_=cosT[:, ss])
                    sin_t = cspool.tile([128, 512], F32, tag="sin")
                    nc.sync.dma_start(out=sin_t, in_=sinTs[:, ss])

                    def rope(dst, src_ps):
                        # ACT copies release the PSUM bank fast (the next seq
                        # block's matmuls wait on it) and produce both the
                        # straight and half-rotated views, so every DVE op is
                        # partition-aligned (cross-partition SBUF operands are
                        # rejected by the BIR verifier; PSUM->SBUF offset
                        # copies are fine)
                        v = ropetmp.tile([128, 512], F32, tag="v", name="v")
                        vr = ropetmp.tile([128, 512], F32, tag="vr", name="vr")
                        nc.scalar.copy(v, src_ps)
                        nc.scalar.copy(vr[0:64, :], src_ps[64:128, :])
                        nc.scalar.copy(vr[64:128, :], src_ps[0:64, :])
                        t = ropetmp.tile([128, 512], F32, tag="t", name="t")
                        u = ropetmp.tile([128, 512], F32, tag="u", name="u")
                        nc.vector.tensor_mul(t, v, cos_t)
                        nc.vector.tensor_mul(u, vr, sin_t)
                        nc.vector.tensor_add(dst, t, u)

                    # V^T -> V via PE transposes (ACT evicts, keeping DVE free
                    # for RoPE; issued first so the PE transposes overlap ropes)
                    vt_sb = vtb.tile([128, 512], F32R, tag="vt")
                    nc.scalar.copy(vt_sb, v_ps)
                    for j in range(4):
                        vt_ps = psVT.tile([128, 128], F32R, tag="vtp", name="vtp")
                        nc.tensor.transpose(
                            vt_ps, vt_sb[:, j * 128:(j + 1) * 128], ident_sb
                        )
                        nc.scalar.copy(vN[:, sb * 4 + j, :], vt_ps)

                    for h in range(QH):
                        rope(qT[:, h, ss], q_ps[h])
                    rope(kT[:, ss], k_ps)

            # ---------------- Phase B/C: attention + out projection ----------------
            with (
                tc.tile_pool(name="wopool", bufs=1) as wopool,
                tc.tile_pool(name="expp", bufs=8) as expp,
                tc.tile_pool(name="otp", bufs=2) as otp,
                tc.tile_pool(name="dsmall", bufs=2) as dsmall,
                tc.tile_pool(name="bcp", bufs=2) as bcp,
                tc.tile_pool(name="outev", bufs=3) as outev,
                tc.tile_pool(name="psS", bufs=2, space="PSUM") as psS,
                tc.tile_pool(name="psD", bufs=2, space="PSUM") as psD,
                tc.tile_pool(name="psOT", bufs=2, space="PSUM") as psOT,
                tc.tile_pool(name="psC", bufs=2, space="PSUM") as psC,
            ):
                wo_sb = wopool.tile([128, QH, DIM], F32R)
                wo_r = wo.rearrange("(h p) n -> p h n", p=128)
                for h in range(QH):
                    for c in range(2):
                        nc.sync.dma_start(
                            out=wo_sb[:, h, c * 2048:(c + 1) * 2048],
                            in_=wo_r[:, h, c * 2048:(c + 1) * 2048],
                        )

                LAG = 4  # D/AV matmuls trail the score stream by LAG blocks
                for qb in range(NSB):
                    qs = slice(qb * 512, (qb + 1) * 512)
                    n_kb = 4 * qb + 4
                    ot_sb = [None] * QH
                    for h in range(QH):
                        d_ps = psD.tile([1, 512], F32, tag="dps", name="dps")
                        ot_ps = psOT.tile([128, 512], F32, tag="otps", name="otps")
                        ess = [None] * n_kb

                        def drain(kb):
                            nc.tensor.matmul(
                                d_ps, onesc_sb, ess[kb],
                                start=(kb == 0), stop=(kb == n_kb - 1),
                            )
                            nc.tensor.matmul(
                                ot_ps, vN[:, kb, :], ess[kb],
                                start=(kb == 0), stop=(kb == n_kb - 1),
                            )

                        for kb in range(n_kb):
                            s_ps = psS.tile([128, 512], F32, tag="sps", name="sps")
                            nc.tensor.matmul(
                                s_ps,
                                kT[:, kb * 128:(kb + 1) * 128],
                                qT[:, h, qs],
                                start=True, stop=True,
                            )
                            j = kb - 4 * qb
                            if j >= 0:  # diagonal block: causal mask
                                nc.vector.tensor_add(
                                    s_ps, s_ps,
                                    stair_sb[:, 384 - 128 * j:896 - 128 * j],
                                )
                            es = expp.tile([128, 512], F32R, tag="es", name="es")
                            nc.scalar.activation(
                                es, s_ps, mybir.ActivationFunctionType.Exp,
                                scale=SCALE,
                            )
                            ess[kb] = es
                            if kb >= LAG:
                                drain(kb - LAG)
                        for kb in range(max(0, n_kb - LAG), n_kb):
                            drain(kb)
                        # normalize: O^T * (1/D) broadcast across partitions
                        rd = dsmall.tile([1, 512], F32R, tag="rd", name="rd")
                        with nc.allow_low_precision("f32r reciprocal for PE bcast"):
                            nc.vector.reciprocal(rd, d_ps)
                        bc_ps = psS.tile([128, 512], F32, tag="sps", name="bc")
                        nc.tensor.matmul(
                            bc_ps, onesr_sb, rd, start=True, stop=True
                        )
                        bc_sb = bcp.tile([128, 512], F32, tag="bcsb", name="bcsb")
                        nc.scalar.copy(bc_sb, bc_ps)
                        ot = otp.tile([128, 512], F32R, tag=f"ot{h}", name=f"ot{h}")
                        nc.vector.tensor_mul(ot, ot_ps, bc_sb)
                        ot_sb[h] = ot

                    # Phase C for this query block
                    for qc in range(4):
                        for nb in range(8):
                            o_ps = psC.tile([128, 512], F32, tag="ops", name="ops")
                            for h in range(QH):
                                nc.tensor.matmul(
                                    o_ps,
                                    ot_sb[h][:, qc * 128:(qc + 1) * 128],
                                    wo_sb[:, h, nb * 512:(nb + 1) * 512],
                                    start=(h == 0), stop=(h == QH - 1),
                                )
                            ob = outev.tile([128, 512], F32, tag="ob", name="ob")
                            nc.vector.tensor_copy(ob, o_ps)
                            nc.sync.dma_start(
                                out=out[qb * 512 + qc * 128:
                                        qb * 512 + (qc + 1) * 128,
                                        nb * 512:(nb + 1) * 512],
                                in_=ob,
                            )
    nc.finalize()
    return nc


_NC_CACHE = {}


def _get_nc():
    if "nc" not in _NC_CACHE:
        _NC_CACHE["nc"] = build_nc()
    return _NC_CACHE["nc"]


def _host_prep(x, cos, sin, mask, wq, wk, wv, wo):
    xT = np.ascontiguousarray(x[0].T.astype(np.float32))
    cosT = np.ascontiguousarray(cos[:, 0, :].T.astype(np.float32))
    sinT = sin[:, 0, :].T.astype(np.float32)
    sinTs = np.ascontiguousarray(
        np.concatenate([-sinT[:64], sinT[64:]], axis=0)
    )
    rr = np.arange(128, dtype=np.int64)[:, None]
    cc = np.arange(896, dtype=np.int64)[None, :]
    stair = np.where(rr <= cc - 384, 0.0, NEG).astype(np.float32)
    ident = np.eye(128, dtype=np.float32)
    ones_col = np.ones((128, 1), dtype=np.float32)
    ones_row = np.ones((1, 128), dtype=np.float32)

    in_maps = []
    for i in range(N_CORES):
        in_maps.append({
            "xT": xT,
            "wq": np.ascontiguousarray(wq[:, i * QS:(i + 1) * QS]),
            "wk": np.ascontiguousarray(wk[:, i * 128:(i + 1) * 128]),
            "wv": np.ascontiguousarray(wv[:, i * 128:(i + 1) * 128]),
            "wo": np.ascontiguousarray(wo[i * QS:(i + 1) * QS, :]),
            "cosT": cosT,
            "sinTs": sinTs,
            "stair": stair,
            "ident": ident,
            "ones_col": ones_col,
            "ones_row": ones_row,
        })
    return in_maps


def kernel(x, cos, sin, mask, wq, wk, wv, wo, _trace=False, _trace_kwargs=None):
    nc = _get_nc()
    in_maps = _host_prep(x, cos, sin, mask, wq, wk, wv, wo)
    res = run_bass_kernel_spmd(
        nc, in_maps, list(range(N_CORES)), trace=_trace,
        **(_trace_kwargs or {}),
    )
    partials = [res.results[i]["out"] for i in range(N_CORES)]
    full = np.sum(np.stack(partials, axis=0), axis=0, dtype=np.float64)
    out = full.astype(np.float32)[None, :, :]
    if _trace:
        return out, res
    return out



# revision 5
# speedup vs baseline: 1.4499x; 1.4499x over previous
"""GQA attention (SEQ=2048, DIM=4096, 32 Q heads / 8 KV heads, head_dim=128),
tensor-parallel over heads across 8 NeuronCores.

Each core owns 4 Q heads + 1 KV head: wq/wk/wv split column-wise, wo split
row-wise; each core produces a partial (2048, 4096) output that the host sums
(the all-reduce of row-parallel wo).

Per-core kernel, v2 (all matmul operands bf16, fp32 PSUM accumulate):
  A) QKV projections per 512-seq block: x slab resident in SBUF (chunked
     10-deep rotation), two passes over the slab (q0,q1,k then q2,q3,v) so
     only 3 PSUM accumulators are live per pass and the 2-buf rotation never
     stalls on eviction; RoPE on PSUM eviction (ACT half-swap + DVE mul/add);
     V^T -> V via PE transposes.
  B) Flash-style attention per (head, 512-query block): S^T = K^T.T @ Q^T,
     causal staircase mask on diagonal blocks, exp on ACT (scale folded),
     D broadcast across partitions via all-ones stationary matmul, O^T
     accumulated in PSUM; normalize = fast-approx reciprocal + mul on DVE
     (entirely off the PE critical path).
  C) out = O^T.T @ wo accumulated over the 4 heads; issued interleaved with
     phase B (B0,B1,C0,B2,C1,B3,C2,C3) so normalize tails hide under B work.

One unified 4-tag x 2-buf PSUM pool (exactly 8 banks) spans all phases to
avoid pool-boundary drain stalls.
"""

import numpy as np
import ml_dtypes

import concourse.bacc as bacc
import concourse.tile as tile
from concourse import mybir
from concourse.bass_utils import run_bass_kernel_spmd

F32 = mybir.dt.float32
F32R = mybir.dt.float32r
BF16 = mybir.dt.bfloat16
NPBF = ml_dtypes.bfloat16

DIM = 4096
SEQ = 2048
HEAD_DIM = 128
N_CORES = 8
QH = 4              # q heads per core
QS = QH * HEAD_DIM  # 512: wq column slice per core
NKT = DIM // 128    # 32 contraction tiles
NSB = SEQ // 512    # 4 sequence blocks
SCALE = 1.0 / float(np.sqrt(HEAD_DIM))
NEG = -1e9
LAG = 4             # D/AV matmuls trail the score stream by LAG blocks


def build_nc():
    nc = bacc.Bacc(trn_type="TRN2")

    xT = nc.declare_dram_parameter("xT", [DIM, SEQ], BF16, isOutput=False)
    wq = nc.declare_dram_parameter("wq", [DIM, QS], BF16, isOutput=False)
    wk = nc.declare_dram_parameter("wk", [DIM, HEAD_DIM], BF16, isOutput=False)
    wv = nc.declare_dram_parameter("wv", [DIM, HEAD_DIM], BF16, isOutput=False)
    wo = nc.declare_dram_parameter("wo", [QS, DIM], BF16, isOutput=False)
    cosT = nc.declare_dram_parameter("cosT", [HEAD_DIM, SEQ], F32, isOutput=False)
    sinTs = nc.declare_dram_parameter("sinTs", [HEAD_DIM, SEQ], F32, isOutput=False)
    stair = nc.declare_dram_parameter("stair", [128, 896], F32, isOutput=False)
    ident = nc.declare_dram_parameter("ident", [128, 128], F32R, isOutput=False)
    ones128 = nc.declare_dram_parameter("ones128", [128, 128], BF16, isOutput=False)
    out = nc.declare_dram_parameter("out", [SEQ, DIM], F32, isOutput=True)

    xT_r = xT.rearrange("(t p) s -> p t s", p=128)
    wq_r = wq.rearrange("(t p) m -> p t m", p=128)
    wk_r = wk.rearrange("(t p) m -> p t m", p=128)
    wv_r = wv.rearrange("(t p) m -> p t m", p=128)
    wo_r = wo.rearrange("(h p) n -> p h n", p=128)

    with tile.TileContext(nc) as tc:
        with (
            tc.tile_pool(name="persist", bufs=1) as persist,
            tc.tile_pool(name="xc", bufs=10) as xcp,
            tc.tile_pool(name="cs", bufs=2) as csp,
            tc.tile_pool(name="rtmp", bufs=2) as rtp,
            tc.tile_pool(name="vtsb", bufs=2) as vtp,
            tc.tile_pool(name="esp", bufs=8) as esp,
            tc.tile_pool(name="otp", bufs=2) as otp,
            tc.tile_pool(name="rdp", bufs=2) as rdp,
            tc.tile_pool(name="obp", bufs=4) as obp,
            tc.tile_pool(name="ps", bufs=2, space="PSUM") as ps,
        ):
            # small constants
            stair_sb = persist.tile([128, 896], F32)
            nc.sync.dma_start(out=stair_sb, in_=stair[:, :])
            ident_f32r = persist.tile([128, 128], F32R)
            nc.sync.dma_start(out=ident_f32r, in_=ident[:, :])
            ones_sb = persist.tile([128, 128], BF16)
            nc.sync.dma_start(out=ones_sb, in_=ones128[:, :])

            # resident weights
            wq_sb = persist.tile([128, NKT, QS], BF16)
            wk_sb = persist.tile([128, NKT, HEAD_DIM], BF16)
            wv_sb = persist.tile([128, NKT, HEAD_DIM], BF16)
            wo_sb = persist.tile([128, QH, DIM], BF16)

            # per-seq-block activation outputs (separate tiles so phase B's
            # dependencies are per-block, not whole-tensor)
            qTb = [persist.tile([128, QH, 512], BF16, name=f"qTb{sb}")
                   for sb in range(NSB)]
            kTb = [persist.tile([128, 512], BF16, name=f"kTb{sb}")
                   for sb in range(NSB)]
            vNb = [persist.tile([128, 4, 128], BF16, name=f"vNb{sb}")
                   for sb in range(NSB)]

            def rope(dst, src_ps, cos_t, sin_t):
                # half-swap via ACT (PSUM->SBUF cross-partition copies are
                # allowed); muls/add on DVE. sin_t already has the rotate_half
                # sign folded in (rows 0:64 negated on host).
                vr = rtp.tile([128, 512], F32, tag="vr", name="vr")
                nc.scalar.copy(vr[0:64, :], src_ps[64:128, :])
                nc.scalar.copy(vr[64:128, :], src_ps[0:64, :])
                u = rtp.tile([128, 512], F32, tag="u", name="u")
                nc.vector.tensor_mul(u, vr, sin_t)
                t2 = rtp.tile([128, 512], F32, tag="t2", name="t2")
                nc.vector.tensor_mul(t2, src_ps, cos_t)
                nc.vector.tensor_add(dst, t2, u)

            # ---------------- Phase A: projections + RoPE ----------------
            for sb in range(NSB):
                ss = slice(sb * 512, (sb + 1) * 512)
                cos_t = csp.tile([128, 512], F32, tag="cos", name="cos")
                nc.sync.dma_start(out=cos_t, in_=cosT[:, ss])
                sin_t = csp.tile([128, 512], F32, tag="sin", name="sin")
                nc.sync.dma_start(out=sin_t, in_=sinTs[:, ss])

                xcs = []
                for c in range(8):
                    if sb == 0:
                        # interleave weight chunks with the x stream so the
                        # first matmuls only queue behind a small DMA prefix
                        nc.sync.dma_start(
                            out=wq_sb[:, c * 4:(c + 1) * 4, :],
                            in_=wq_r[:, c * 4:(c + 1) * 4, :],
                        )
                        nc.sync.dma_start(
                            out=wk_sb[:, c * 4:(c + 1) * 4, :],
                            in_=wk_r[:, c * 4:(c + 1) * 4, :],
                        )
                        nc.sync.dma_start(
                            out=wv_sb[:, c * 4:(c + 1) * 4, :],
                            in_=wv_r[:, c * 4:(c + 1) * 4, :],
                        )
                    xc = xcp.tile([128, 4, 512], BF16, tag="xc",
                                  name=f"xc{sb}_{c}")
                    nc.sync.dma_start(out=xc, in_=xT_r[:, c * 4:(c + 1) * 4, ss])
                    xcs.append(xc)
                # wo prefetch: one 1 MiB chunk per sb, after the x chunks
                nc.sync.dma_start(out=wo_sb[:, sb, :], in_=wo_r[:, sb, :])

                # pass 1: q heads 0,1 + K
                a_ps = ps.tile([128, 512], F32, tag="a", name="aps")
                b_ps = ps.tile([128, 512], F32, tag="b", name="bps")
                c_ps = ps.tile([128, 512], F32, tag="c", name="cps")
                for c in range(8):
                    for i in range(4):
                        kt = c * 4 + i
                        st, sp = (kt == 0), (kt == NKT - 1)
                        xt = xcs[c][:, i, :]
                        nc.tensor.matmul(a_ps, wq_sb[:, kt, 0:128], xt,
                                         start=st, stop=sp)
                        nc.tensor.matmul(b_ps, wq_sb[:, kt, 128:256], xt,
                                         start=st, stop=sp)
                        nc.tensor.matmul(c_ps, wk_sb[:, kt, :], xt,
                                         start=st, stop=sp)
                rope(qTb[sb][:, 0, :], a_ps, cos_t, sin_t)
                rope(qTb[sb][:, 1, :], b_ps, cos_t, sin_t)
                rope(kTb[sb], c_ps, cos_t, sin_t)

                # pass 2: q heads 2,3 + V
                d_ps = ps.tile([128, 512], F32, tag="a", name="dps")
                e_ps = ps.tile([128, 512], F32, tag="b", name="eps")
                f_ps = ps.tile([128, 512], F32, tag="c", name="fps")
                for c in range(8):
                    for i in range(4):
                        kt = c * 4 + i
                        st, sp = (kt == 0), (kt == NKT - 1)
                        xt = xcs[c][:, i, :]
                        nc.tensor.matmul(d_ps, wq_sb[:, kt, 256:384], xt,
                                         start=st, stop=sp)
                        nc.tensor.matmul(e_ps, wq_sb[:, kt, 384:512], xt,
                                         start=st, stop=sp)
                        nc.tensor.matmul(f_ps, wv_sb[:, kt, :], xt,
                                         start=st, stop=sp)
                # V^T -> V via PE transposes first (overlaps the ropes)
                vt_sb = vtp.tile([128, 512], F32R, tag="vt", name="vt")
                nc.scalar.copy(vt_sb, f_ps)
                for j in range(4):
                    vt_ps = ps.tile([128, 128], F32R, tag="t", name="vtp")
                    nc.tensor.transpose(
                        vt_ps, vt_sb[:, j * 128:(j + 1) * 128], ident_f32r
                    )
                    nc.scalar.copy(vNb[sb][:, j, :], vt_ps)
                rope(qTb[sb][:, 2, :], d_ps, cos_t, sin_t)
                rope(qTb[sb][:, 3, :], e_ps, cos_t, sin_t)

            # ---------------- Phase B: attention per query block ----------------
            ots = [[None] * QH, [None] * QH]  # double-buffered across qb

            def attention(qb):
                n_kb = 4 * qb + 4
                for h in range(QH):
                    d_ps = ps.tile([128, 512], F32, tag="b", name="dattn")
                    ot_ps = ps.tile([128, 512], F32, tag="c", name="otps")
                    ess = [None] * n_kb

                    def drain(kb):
                        st, sp = (kb == 0), (kb == n_kb - 1)
                        # D broadcast across partitions: all-ones stationary
                        nc.tensor.matmul(d_ps, ones_sb, ess[kb],
                                         start=st, stop=sp)
                        nc.tensor.matmul(ot_ps, vNb[kb // 4][:, kb % 4, :],
                                         ess[kb], start=st, stop=sp)

                    for kb in range(n_kb):
                        s_ps = ps.tile([128, 512], F32, tag="a", name="sps")
                        nc.tensor.matmul(
                            s_ps,
                            kTb[kb // 4][:, (kb % 4) * 128:(kb % 4 + 1) * 128],
                            qTb[qb][:, h, :],
                            start=True, stop=True,
                        )
                        j = kb - 4 * qb
                        if j >= 0:  # diagonal block: causal staircase mask
                            nc.vector.tensor_add(
                                s_ps, s_ps,
                                stair_sb[:, 384 - 128 * j:896 - 128 * j],
                            )
                        es = esp.tile([128, 512], BF16, tag="es", name="es")
                        nc.scalar.activation(
                            es, s_ps, mybir.ActivationFunctionType.Exp,
                            scale=SCALE,
                        )
                        ess[kb] = es
                        if kb >= LAG:
                            drain(kb - LAG)
                    for kb in range(max(0, n_kb - LAG), n_kb):
                        drain(kb)
                    # normalize entirely on DVE, off the PE critical path
                    rd = rdp.tile([128, 512], F32, tag="rd", name="rd")
                    nc.vector.reciprocal_approx_fast(rd, d_ps)
                    ot = otp.tile([128, 512], BF16, tag=f"ot{h}", name=f"ot{h}")
                    nc.vector.tensor_mul(ot, ot_ps, rd)
                    ots[qb % 2][h] = ot

            # ---------------- Phase C: output projection ----------------
            def outproj(qb):
                ot_sb = ots[qb % 2]
                for qc in range(4):
                    for nb in range(8):
                        o_ps = ps.tile([128, 512], F32, tag="t", name="ops")
                        for h in range(QH):
                            nc.tensor.matmul(
                                o_ps,
                                ot_sb[h][:, qc * 128:(qc + 1) * 128],
                                wo_sb[:, h, nb * 512:(nb + 1) * 512],
                                start=(h == 0), stop=(h == QH - 1),
                            )
                        ob = obp.tile([128, 512], F32, tag="ob", name="ob")
                        # alternate eviction engine to halve per-engine load
                        if (qc * 8 + nb) % 2 == 0:
                            nc.vector.tensor_copy(ob, o_ps)
                        else:
                            nc.scalar.copy(ob, o_ps)
                        nc.sync.dma_start(
                            out=out[qb * 512 + qc * 128:
                                    qb * 512 + (qc + 1) * 128,
                                    nb * 512:(nb + 1) * 512],
                            in_=ob,
                        )

            # software pipeline: C(qb) issued after B(qb+1) so the normalize
            # tail of B(qb) hides under B(qb+1)'s score stream
            attention(0)
            attention(1)
            outproj(0)
            attention(2)
            outproj(1)
            attention(3)
            outproj(2)
            outproj(3)
    nc.finalize()
    return nc


_NC_CACHE = {}


def _get_nc():
    if "nc" not in _NC_CACHE:
        _NC_CACHE["nc"] = build_nc()
    return _NC_CACHE["nc"]


def _host_prep(x, cos, sin, mask, wq, wk, wv, wo):
    xT = np.ascontiguousarray(x[0].T.astype(np.float32)).astype(NPBF)
    cosT = np.ascontiguousarray(cos[:, 0, :].T.astype(np.float32))
    sinT = sin[:, 0, :].T.astype(np.float32)
    sinTs = np.ascontiguousarray(
        np.concatenate([-sinT[:64], sinT[64:]], axis=0)
    )
    rr = np.arange(128, dtype=np.int64)[:, None]
    cc = np.arange(896, dtype=np.int64)[None, :]
    stair = np.where(rr <= cc - 384, 0.0, NEG).astype(np.float32)
    ident = np.eye(128, dtype=np.float32)
    ones128 = np.ones((128, 128), dtype=np.float32).astype(NPBF)

    in_maps = []
    for i in range(N_CORES):
        in_maps.append({
            "xT": xT,
            "wq": np.ascontiguousarray(wq[:, i * QS:(i + 1) * QS]).astype(NPBF),
            "wk": np.ascontiguousarray(wk[:, i * 128:(i + 1) * 128]).astype(NPBF),
            "wv": np.ascontiguousarray(wv[:, i * 128:(i + 1) * 128]).astype(NPBF),
            "wo": np.ascontiguousarray(wo[i * QS:(i + 1) * QS, :]).astype(NPBF),
            "cosT": cosT,
            "sinTs": sinTs,
            "stair": stair,
            "ident": ident,
            "ones128": ones128,
        })
    return in_maps


def kernel(x, cos, sin, mask, wq, wk, wv, wo, _trace=False, _trace_kwargs=None):
    nc = _get_nc()
    in_maps = _host_prep(x, cos, sin, mask, wq, wk, wv, wo)
    res = run_bass_kernel_spmd(
        nc, in_maps, list(range(N_CORES)), trace=_trace,
        **(_trace_kwargs or {}),
    )
    partials = [res.results[i]["out"] for i in range(N_CORES)]
    full = np.sum(np.stack(partials, axis=0), axis=0, dtype=np.float64)
    out = full.astype(np.float32)[None, :, :]
    if _trace:
        return out, res
    return out


# revision 12
# speedup vs baseline: 1.5046x; 1.0377x over previous
"""GQA attention (SEQ=2048, DIM=4096, 32 Q heads / 8 KV heads, head_dim=128),
tensor-parallel over heads across 8 NeuronCores.

Each core owns 4 Q heads + 1 KV head: wq/wk/wv split column-wise, wo split
row-wise; each core produces a partial (2048, 4096) output that the host sums
(the all-reduce of row-parallel wo).

Per-core kernel, v3 (all matmul operands bf16, fp32 PSUM accumulate):
  A) QKV projections per 512-seq block: x slab resident in SBUF (chunked
     9-deep rotation), two passes over the slab (q0,q1,K then q2,q3,V) so only
     3 PSUM accumulators are live per pass and the 2-buf rotation never stalls
     on eviction; RoPE on PSUM eviction (ACT half-swap + DVE mul/add). V^T is
     evicted to SBUF; its PE transposes to natural layout are deferred to the
     matching phase-B header so they never stall the projection stream.
  B) Flash attention, flattened over (head, key-block) per 512-query block:
     score pairs go into wide [128,1024] PSUM tiles (one exp covers 2 blocks,
     halving ACT instruction overhead); diagonal blocks are causally trimmed
     (S/exp/D/AV only touch queries >= the block diagonal), with the j=0
     diagonal drained last so the PSUM accumulation stop lands on a full-range
     matmul; D is broadcast across partitions via an all-ones stationary;
     D/AV drains trail the score stream by LAG=6 blocks across head
     boundaries; normalize = fast-approx reciprocal + mul on DVE.
  C) out = O^T.T @ wo accumulated over the 4 heads, bf16 partials to DRAM;
     issued interleaved with phase B (B0,B1,C0,B2,C1,B3,C2,C3).

One unified PSUM pool: s2 [128,1024]x2 + b,c [128,512]x2 = exactly 8 banks,
spanning all phases (no pool-boundary drain stalls). Phase C's o_ps shares
tag b with phase B's D accumulators; phase A's accumulators share s2/b/c.
"""

import numpy as np
import ml_dtypes

import concourse.bacc as bacc
import concourse.tile as tile
from concourse import mybir
from concourse.bass_utils import run_bass_kernel_spmd

F32 = mybir.dt.float32
F32R = mybir.dt.float32r
BF16 = mybir.dt.bfloat16
NPBF = ml_dtypes.bfloat16

DIM = 4096
SEQ = 2048
HEAD_DIM = 128
N_CORES = 8
QH = 4              # q heads per core
QS = QH * HEAD_DIM  # 512: wq column slice per core
NKT = DIM // 128    # 32 contraction tiles
NSB = SEQ // 512    # 4 sequence blocks
SCALE = 1.0 / float(np.sqrt(HEAD_DIM))
NEG = -1e9
LAG = 6             # D/AV drains trail the score stream by LAG blocks


def build_nc():
    nc = bacc.Bacc(trn_type="TRN2")

    xT = nc.declare_dram_parameter("xT", [DIM, SEQ], BF16, isOutput=False)
    wq = nc.declare_dram_parameter("wq", [DIM, QS], BF16, isOutput=False)
    wk = nc.declare_dram_parameter("wk", [DIM, HEAD_DIM], BF16, isOutput=False)
    wv = nc.declare_dram_parameter("wv", [DIM, HEAD_DIM], BF16, isOutput=False)
    wo = nc.declare_dram_parameter("wo", [QS, DIM], BF16, isOutput=False)
    cosT = nc.declare_dram_parameter("cosT", [HEAD_DIM, SEQ], F32, isOutput=False)
    sinTs = nc.declare_dram_parameter("sinTs", [HEAD_DIM, SEQ], F32, isOutput=False)
    stair = nc.declare_dram_parameter("stair", [128, 128], F32, isOutput=False)
    stair9 = nc.declare_dram_parameter("stair9", [128, 896], F32, isOutput=False)
    ident = nc.declare_dram_parameter("ident", [128, 128], F32R, isOutput=False)
    ones128 = nc.declare_dram_parameter("ones128", [128, 128], BF16, isOutput=False)
    out = nc.declare_dram_parameter("out", [SEQ, DIM], BF16, isOutput=True)

    xT_r = xT.rearrange("(t p) s -> p t s", p=128)
    wq_r = wq.rearrange("(t p) m -> p t m", p=128)
    wk_r = wk.rearrange("(t p) m -> p t m", p=128)
    wv_r = wv.rearrange("(t p) m -> p t m", p=128)
    wo_r = wo.rearrange("(h p) n -> p h n", p=128)

    with tile.TileContext(nc) as tc:
        with (
            tc.tile_pool(name="persist", bufs=1) as persist,
            tc.tile_pool(name="xc", bufs=9) as xcp,
            tc.tile_pool(name="cs", bufs=2) as csp,
            tc.tile_pool(name="rtmp", bufs=2) as rtp,
            tc.tile_pool(name="vtsb", bufs=4) as vtp,
            tc.tile_pool(name="esp", bufs=6) as esp,
            tc.tile_pool(name="otp", bufs=2) as otp,
            tc.tile_pool(name="rdp", bufs=2) as rdp,
            tc.tile_pool(name="obp", bufs=4) as obp,
            tc.tile_pool(name="ps", bufs=2, space="PSUM") as ps,
        ):
            # small constants
            stair_sb = persist.tile([128, 128], F32)
            nc.sync.dma_start(out=stair_sb, in_=stair[:, :])
            stair9_sb = persist.tile([128, 896], F32)
            nc.sync.dma_start(out=stair9_sb, in_=stair9[:, :])
            ident_f32r = persist.tile([128, 128], F32R)
            nc.sync.dma_start(out=ident_f32r, in_=ident[:, :])
            ones_sb = persist.tile([128, 128], BF16)
            nc.sync.dma_start(out=ones_sb, in_=ones128[:, :])

            # resident weights
            wq_sb = persist.tile([128, NKT, QS], BF16)
            wk_sb = persist.tile([128, NKT, HEAD_DIM], BF16)
            wv_sb = persist.tile([128, NKT, HEAD_DIM], BF16)
            wo_sb = persist.tile([128, QH, DIM], BF16)

            # per-seq-block activation outputs
            qTb = [persist.tile([128, QH, 512], BF16, name=f"qTb{sb}")
                   for sb in range(NSB)]
            kTb = [persist.tile([128, 512], BF16, name=f"kTb{sb}")
                   for sb in range(NSB)]
            vNb = [persist.tile([128, 4, 128], BF16, name=f"vNb{sb}")
                   for sb in range(NSB)]
            vt_sbs = [None] * NSB  # V^T staged in SBUF, transposed in B headers

            def rope(dst, src_ps, cos_t, sin_t):
                # half-swap via ACT (PSUM->SBUF cross-partition copies are
                # allowed); muls/add on DVE. sin_t has the rotate_half sign
                # folded in (rows 0:64 negated on host).
                vr = rtp.tile([128, 512], F32, tag="vr", name="vr")
                nc.scalar.copy(vr[0:64, :], src_ps[64:128, :])
                nc.scalar.copy(vr[64:128, :], src_ps[0:64, :])
                u = rtp.tile([128, 512], F32, tag="u", name="u")
                nc.vector.tensor_mul(u, vr, sin_t)
                t2 = rtp.tile([128, 512], F32, tag="t2", name="t2")
                nc.vector.tensor_mul(t2, src_ps, cos_t)
                nc.vector.tensor_add(dst, t2, u)

            # ---------------- Phase A: projections + RoPE ----------------
            for sb in range(NSB):
                ss = slice(sb * 512, (sb + 1) * 512)
                cos_t = csp.tile([128, 512], F32, tag="cos", name="cos")
                nc.sync.dma_start(out=cos_t, in_=cosT[:, ss])
                sin_t = csp.tile([128, 512], F32, tag="sin", name="sin")
                nc.sync.dma_start(out=sin_t, in_=sinTs[:, ss])

                xcs = []
                for c in range(8):
                    if sb == 0:
                        nc.sync.dma_start(
                            out=wq_sb[:, c * 4:(c + 1) * 4, :],
                            in_=wq_r[:, c * 4:(c + 1) * 4, :],
                        )
                        nc.sync.dma_start(
                            out=wk_sb[:, c * 4:(c + 1) * 4, :],
                            in_=wk_r[:, c * 4:(c + 1) * 4, :],
                        )
                        nc.sync.dma_start(
                            out=wv_sb[:, c * 4:(c + 1) * 4, :],
                            in_=wv_r[:, c * 4:(c + 1) * 4, :],
                        )
                    xc = xcp.tile([128, 4, 512], BF16, tag="xc",
                                  name=f"xc{sb}_{c}")
                    nc.sync.dma_start(out=xc, in_=xT_r[:, c * 4:(c + 1) * 4, ss])
                    xcs.append(xc)
                nc.sync.dma_start(out=wo_sb[:, sb, :], in_=wo_r[:, sb, :])

                # pass 1: q heads 0,1 + K
                a_ps = ps.tile([128, 512], F32, tag="s2", name="aps")
                b_ps = ps.tile([128, 512], F32, tag="b", name="bps")
                c_ps = ps.tile([128, 512], F32, tag="c", name="cps")
                for c in range(8):
                    for i in range(4):
                        kt = c * 4 + i
                        st, sp = (kt == 0), (kt == NKT - 1)
                        xt = xcs[c][:, i, :]
                        nc.tensor.matmul(a_ps, wq_sb[:, kt, 0:128], xt,
                                         start=st, stop=sp)
                        nc.tensor.matmul(b_ps, wq_sb[:, kt, 128:256], xt,
                                         start=st, stop=sp)
                        nc.tensor.matmul(c_ps, wk_sb[:, kt, :], xt,
                                         start=st, stop=sp)
                rope(qTb[sb][:, 0, :], a_ps, cos_t, sin_t)
                rope(qTb[sb][:, 1, :], b_ps, cos_t, sin_t)
                rope(kTb[sb], c_ps, cos_t, sin_t)

                # pass 2: q heads 2,3 + V
                d_ps = ps.tile([128, 512], F32, tag="s2", name="dps")
                e_ps = ps.tile([128, 512], F32, tag="b", name="eps")
                f_ps = ps.tile([128, 512], F32, tag="c", name="fps")
                for c in range(8):
                    for i in range(4):
                        kt = c * 4 + i
                        st, sp = (kt == 0), (kt == NKT - 1)
                        xt = xcs[c][:, i, :]
                        nc.tensor.matmul(d_ps, wq_sb[:, kt, 256:384], xt,
                                         start=st, stop=sp)
                        nc.tensor.matmul(e_ps, wq_sb[:, kt, 384:512], xt,
                                         start=st, stop=sp)
                        nc.tensor.matmul(f_ps, wv_sb[:, kt, :], xt,
                                         start=st, stop=sp)
                vt_sb = vtp.tile([128, 512], F32R, tag="vt", name=f"vt{sb}")
                nc.scalar.copy(vt_sb, f_ps)
                vt_sbs[sb] = vt_sb
                rope(qTb[sb][:, 2, :], d_ps, cos_t, sin_t)
                rope(qTb[sb][:, 3, :], e_ps, cos_t, sin_t)

            # ---------------- Phase B: attention per query block ----------------
            ots = [[None] * QH, [None] * QH]  # double-buffered across qb

            def attention(qb):
                # header: V transposes for this qb's diagonal KV tile (their
                # inputs have been ready since phase A; zero-stall PE work)
                for j in range(4):
                    vt_ps = ps.tile([128, 128], F32R, tag="s2", name="vtp")
                    nc.tensor.transpose(
                        vt_ps, vt_sbs[qb][:, j * 128:(j + 1) * 128], ident_f32r
                    )
                    nc.scalar.copy(vNb[qb][:, j, :], vt_ps)

                n_kb = 4 * qb + 4
                # drain schedule per head: full blocks in order, then diagonal
                # j=1..3 (trimmed), then j=0 last (full range, carries stop).
                # For qb==0 there is no leading full block, so j=0 is split
                # into [0,128) start+stop and [128,512) stop.
                dq = []  # (h, kb, lo, start, stop, head_last)
                for h in range(QH):
                    items = []
                    if qb == 0:
                        # untrimmed, in order: es is exactly 0 in the masked
                        # region (staircase + exp underflow), so full-range
                        # drains with a single leading start are correct
                        for kb in range(4):
                            items.append((kb, 0, kb == 0, kb == 3))
                    else:
                        for kb in range(4 * qb):
                            items.append((kb, 0, kb == 0, False))
                        items.append((4 * qb + 1, 128, False, False))
                        items.append((4 * qb + 2, 256, False, False))
                        items.append((4 * qb + 3, 384, False, False))
                        items.append((4 * qb, 0, False, True))
                    for idx, it in enumerate(items):
                        dq.append((h, it, idx == len(items) - 1))

                d_ps_h = [None] * QH
                ot_ps_h = [None] * QH
                esw = {}
                state = {"dqi": 0, "issued": 0}

                def do_drain():
                    h, (kb, rng, st, sp), head_last = dq[state["dqi"]]
                    state["dqi"] += 1
                    if d_ps_h[h] is None:
                        d_ps_h[h] = ps.tile([128, 512], F32, tag="b",
                                            name=f"dq{qb}_{h}")
                        ot_ps_h[h] = ps.tile([128, 512], F32, tag="c",
                                             name=f"oq{qb}_{h}")
                    lo, hi = rng if isinstance(rng, tuple) else (rng, 512)
                    es = esw[(h, kb // 2)]
                    half = kb % 2
                    mv = es[:, half * 512 + lo: half * 512 + hi]
                    nc.tensor.matmul(d_ps_h[h][:, lo:hi], ones_sb, mv,
                                     start=st, stop=sp)
                    nc.tensor.matmul(ot_ps_h[h][:, lo:hi],
                                     vNb[kb // 4][:, kb % 4, :], mv,
                                     start=st, stop=sp)
                    if head_last:
                        rd = rdp.tile([128, 512], F32, tag="rd", name="rd")
                        nc.vector.reciprocal_approx_fast(rd, d_ps_h[h])
                        ot = otp.tile([128, 512], BF16, tag=f"ot{h}",
                                      name=f"ot{h}")
                        nc.vector.tensor_mul(ot, ot_ps_h[h], rd)
                        ots[qb % 2][h] = ot

                for h in range(QH):
                    for p in range(n_kb // 2):
                        sw = ps.tile([128, 1024], F32, tag="s2", name="sw")
                        for half, kb in enumerate((2 * p, 2 * p + 1)):
                            j = kb - 4 * qb
                            lo = j * 128 if (j > 0 and qb > 0) else 0
                            nc.tensor.matmul(
                                sw[:, half * 512 + lo:(half + 1) * 512],
                                kTb[kb // 4][:, (kb % 4) * 128:
                                             (kb % 4 + 1) * 128],
                                qTb[qb][:, h, lo:512],
                                start=True, stop=True,
                            )
                            if j >= 0:
                                if qb == 0:
                                    # full staircase so masked es is exactly 0
                                    dst = sw[:, half * 512:(half + 1) * 512]
                                    nc.vector.tensor_add(
                                        dst, dst,
                                        stair9_sb[:, 384 - 128 * j:
                                                  896 - 128 * j],
                                    )
                                else:
                                    dst = sw[:, half * 512 + j * 128:
                                             half * 512 + (j + 1) * 128]
                                    nc.vector.tensor_add(dst, dst,
                                                         stair_sb[:, :])
                        jA = (max(0, 2 * p - 4 * qb) * 128) if qb > 0 else 0
                        es = esp.tile([128, 1024], BF16, tag="es", name="es")
                        nc.scalar.activation(
                            es[:, jA:], sw[:, jA:],
                            mybir.ActivationFunctionType.Exp, scale=SCALE,
                        )
                        esw[(h, p)] = es
                        state["issued"] += 2
                        while (state["dqi"] < len(dq)
                               and state["dqi"] < state["issued"] - LAG):
                            do_drain()
                while state["dqi"] < len(dq):
                    do_drain()

            # ---------------- Phase C: output projection ----------------
            def outproj(qb):
                ot_sb = ots[qb % 2]
                for qc in range(4):
                    for nb in range(8):
                        o_ps = ps.tile([128, 512], F32, tag="b", name="ops")
                        for h in range(QH):
                            nc.tensor.matmul(
                                o_ps,
                                ot_sb[h][:, qc * 128:(qc + 1) * 128],
                                wo_sb[:, h, nb * 512:(nb + 1) * 512],
                                start=(h == 0), stop=(h == QH - 1),
                            )
                        ob = obp.tile([128, 512], BF16, tag="ob", name="ob")
                        if (qc * 8 + nb) % 2 == 0:
                            nc.vector.tensor_copy(ob, o_ps)
                        else:
                            nc.scalar.copy(ob, o_ps)
                        nc.sync.dma_start(
                            out=out[qb * 512 + qc * 128:
                                    qb * 512 + (qc + 1) * 128,
                                    nb * 512:(nb + 1) * 512],
                            in_=ob,
                        )

            attention(0)
            attention(1)
            outproj(0)
            attention(2)
            outproj(1)
            attention(3)
            outproj(2)
            outproj(3)
    nc.finalize()
    return nc


_NC_CACHE = {}


def _get_nc():
    if "nc" not in _NC_CACHE:
        _NC_CACHE["nc"] = build_nc()
    return _NC_CACHE["nc"]


def _host_prep(x, cos, sin, mask, wq, wk, wv, wo):
    xT = np.ascontiguousarray(x[0].T.astype(np.float32)).astype(NPBF)
    cosT = np.ascontiguousarray(cos[:, 0, :].T.astype(np.float32))
    sinT = sin[:, 0, :].T.astype(np.float32)
    sinTs = np.ascontiguousarray(
        np.concatenate([-sinT[:64], sinT[64:]], axis=0)
    )
    rr = np.arange(128, dtype=np.int64)[:, None]
    cc = np.arange(128, dtype=np.int64)[None, :]
    stair = np.where(rr <= cc, 0.0, NEG).astype(np.float32)
    cc9 = np.arange(896, dtype=np.int64)[None, :]
    stair9 = np.where(rr <= cc9 - 384, 0.0, NEG).astype(np.float32)
    ident = np.eye(128, dtype=np.float32)
    ones128 = np.ones((128, 128), dtype=np.float32).astype(NPBF)

    in_maps = []
    for i in range(N_CORES):
        in_maps.append({
            "xT": xT,
            "wq": np.ascontiguousarray(wq[:, i * QS:(i + 1) * QS]).astype(NPBF),
            "wk": np.ascontiguousarray(wk[:, i * 128:(i + 1) * 128]).astype(NPBF),
            "wv": np.ascontiguousarray(wv[:, i * 128:(i + 1) * 128]).astype(NPBF),
            "wo": np.ascontiguousarray(wo[i * QS:(i + 1) * QS, :]).astype(NPBF),
            "cosT": cosT,
            "sinTs": sinTs,
            "stair": stair,
            "stair9": stair9,
            "ident": ident,
            "ones128": ones128,
        })
    return in_maps


def kernel(x, cos, sin, mask, wq, wk, wv, wo, _trace=False, _trace_kwargs=None):
    nc = _get_nc()
    in_maps = _host_prep(x, cos, sin, mask, wq, wk, wv, wo)
    res = run_bass_kernel_spmd(
        nc, in_maps, list(range(N_CORES)), trace=_trace,
        **(_trace_kwargs or {}),
    )
    partials = [res.results[i]["out"].astype(np.float64)
                for i in range(N_CORES)]
    full = np.sum(np.stack(partials, axis=0), axis=0)
    out = full.astype(np.float32)[None, :, :]
    if _trace:
        return out, res
    return out
